# revision 41
# baseline (speedup 1.0000x reference)
"""Causal GQA self-attention on 8 Trainium2 NeuronCores (fp16 pipeline).

Sharding: data-parallel over batch (4) x tensor-parallel over heads (2 halves
of 14 heads each, KV heads replicated per GQA group). Each core computes its
heads' partial contribution through the row-parallel out-projection; the host
sums the two fp16 partials per batch element in fp32.

Per-core local structure: 4 local kv groups g (sizes 4,4,4,2 heads), local
head h -> (g = h//4, s = h%4 slot). All tensors fp16 except PSUM (f32).

Layouts (SBUF [128 partitions, free...]):
  xT  [128, 7, 2048]   x^T (C on partitions), DMA'd directly (host pre-transposes)
  QT  [128, 4, 2048]   Q^T: head (g,s) at partitions 32g..32g+32, slot s
  KT  [128, 2048]      K^T: group g at partitions 32g..32g+32
  V   [128, 16, 4, 33] V rows (kpos%128 on partitions), col 32 = ones (Z)
  AO  [128, 4, 448]    attn out rows (q%128 on partitions) per 512-q round
  AOT [128, 4, 2048]   attn out transposed (head dims on partitions)

Pipeline: 4 rounds of 512 tokens; per round: Q/K/V projection, then per head:
S^T = K^T.T @ Q^T per 128-kpos chunk (diagonal chunks get a mask preloaded
into PSUM via an extra matmul, then accumulate), exp greedily load-balanced
across ScalarE (exact), DVE and Pool/GPSIMD (Schraudolph fast-exp: bit-trick
y = s*A+B -> int16 -> reinterpret as fp16), then AV flipped: out[q,d] with
P^T chunk as stationary operand and [V | 1] as 33-wide moving operand so the
softmax denominator Z rides along as column 32. Normalize batched per
(head, round), transpose AO via PE, row-parallel out-projection, fp16
partial out. All elementwise work (exp, PSUM->SBUF copies, normalize) is
assigned per-op to the least-loaded of {Act, DVE, Pool} by a static greedy
cost model.
"""

import sys

sys.path.insert(0, "/opt/trn_rl_repo")

import numpy as np

import concourse.bass as bass
import concourse.mybir as mybir
import concourse.tile as tile
from concourse import bacc
from concourse.bass import ts
from concourse.bass_utils import run_bass_kernel_spmd

F32 = mybir.dt.float32
F16 = mybir.dt.float16
F8 = mybir.dt.float8e4
I16 = mybir.dt.int16
EXP = mybir.ActivationFunctionType.Exp
COPY = mybir.ActivationFunctionType.Copy
DR = mybir.MatmulPerfMode.DoubleRow
MULT = mybir.AluOpType.mult
ADD = mybir.AluOpType.add
P = 128
T, C = 2048, 896
D = 32
HL = 14          # local heads per core
DH = HL * D      # 448
SCALE = 1.0 / float(np.sqrt(D))
# Wq/Wk are pre-scaled x16 on the host so q,k land in fp8e4m3's sweet spot;
# scores come out x256, compensated in the exp scale / Schraudolph slope.
WSCALE = 16.0
SSCALE = SCALE / (WSCALE * WSCALE)
MASKVAL = -180.0 * WSCALE * WSCALE
# Schraudolph fast-exp consts (fp16 bit trick): y = s*A + B as int16
A_S = SSCALE * 1024.0 / float(np.log(2.0))
B_S = 15.0 * 1024.0 - 0.043 * 1024.0

SOFF = [0, 128, 256, 352]   # Wq col offset per slot
SLOTW = [128, 128, 96, 96]  # slot widths (s>=2 lack group 3)

HEADS_HALF = [
    list(range(0, 12)) + [24, 25],
    list(range(12, 24)) + [26, 27],
]
KV_HALF = [[0, 1, 2, 6], [3, 4, 5, 6]]


def _head_gs(h):
    return (h // 4, h % 4) if h < 12 else (3, h - 12)


def _trace(tc_, d):
    nc = tc_.nc

    # greedy elementwise load balancing across Act / DVE / Pool
    eng_load = {"act": 0.0, "dve": 0.0, "pool": 0.0}

    def pick(costs):
        e = min(costs, key=lambda k: eng_load[k] + costs[k])
        eng_load[e] += costs[e]
        return e

    def copy_cost(w, src16=False):
        # f32-PSUM (or f16-PSUM when src16) -> SBUF f16 copy costs
        # (GPSIMD/Pool cannot access PSUM on TRN2, so only Act/DVE here)
        dvec = (0.52 if src16 else 1.0417) * w + 125
        return {"act": 0.833 * w + 185, "dve": dvec}

    def emit_copy(dst, src, src16=False, scale=None):
        e = pick(copy_cost(src.free_size(), src16))
        if e == "act":
            nc.scalar.activation(dst, src, COPY, scale=1.0 if scale is None else scale)
        else:
            if scale is None:
                nc.vector.tensor_copy(dst, src)
            else:
                nc.vector.tensor_scalar_mul(dst, src, scale)

    def emit_exp(pt, sp, n, qoffE):
        w = n * (512 - qoffE)
        e = pick({"act": 0.833 * w + 185,
                  "dve": 1.0417 * w + 125})
        if e == "act":
            nc.scalar.activation(
                pt[:, :, qoffE:512], sp[:, :, qoffE:512], EXP, scale=SSCALE
            )
        else:
            nc.vector.tensor_scalar(
                pt[:, :, qoffE:512].bitcast(I16),
                sp[:, :, qoffE:512], A_S, B_S, MULT, ADD,
            )

    def emit_mask(ap, maskt):
        # zero the strictly-upper (future) triangle of a diagonal 128x128
        # P^T block post-exp; SBUF-only, so the otherwise-idle Pool engine
        # can absorb most of these
        e = pick({"dve": 0.52 * 128 + 60, "pool": 95 + 1.984 * 128})
        tt = nc.vector.tensor_tensor if e == "dve" else nc.gpsimd.tensor_tensor
        tt(ap, ap, maskt[:], MULT)

    with tc_.tile_pool(name="const", bufs=1) as const, \
         tc_.tile_pool(name="persist", bufs=1) as persist, \
         tc_.tile_pool(name="aop", bufs=2) as aop, \
         tc_.tile_pool(name="ptp", bufs=16) as ptp, \
         tc_.tile_pool(name="rzp", bufs=2) as rzp, \
         tc_.tile_pool(name="obp", bufs=2) as obp, \
         tc_.tile_pool(name="spp", bufs=2, space="PSUM") as spp, \
         tc_.tile_pool(name="ppp", bufs=1, space="PSUM") as ppp, \
         tc_.tile_pool(name="avp", bufs=1, space="PSUM") as avp:

        identh = const.tile([P, P], F16)
        maskt = const.tile([P, P], F16)
        warm = const.tile([P, 512], F16)

        xT = persist.tile([P, 7, T], F16, tag="xT")
        # Q8/K8 hold x16-scaled q,k in fp8e4m3 for the DoubleRow S matmul.
        # The second k-tile (index 1) is zeroed once and never written again:
        # DoubleRow contracts over 2 k-tiles, and padding the second with
        # zeros gives a plain 32-deep contraction at 0.5 cycles/col.
        Q8 = persist.tile([P, 4, 2, T], F8, tag="Q8")
        K8 = persist.tile([P, 2, T], F8, tag="K8")
        V = persist.tile([P, 16, 4, 33], F16, tag="V")
        AOT = persist.tile([P, 4, T], F16, tag="AOT")
        WqH = persist.tile([P, 7, DH], F16, tag="WqH")
        WkH = persist.tile([P, 7, P], F16, tag="WkH")
        WvH = persist.tile([P, 7, P], F16, tag="WvH")
        WoH = persist.tile([P, 4, C], F16, tag="WoH")

        xtv = d["xt"].rearrange("(co ci) t -> ci co t", ci=P)
        ov = d["out"].rearrange("(to ti) c -> ti to c", ti=P)

        # input DMAs: round-0 x^T first, then QKV weights, rest of x^T, Wo
        nc.sync.dma_start(xT[:, :, 0:512], xtv[:, :, 0:512])
        nc.sync.dma_start(WqH[:], d["wq"].rearrange("(co ci) n -> ci co n", ci=P))
        nc.sync.dma_start(WkH[:], d["wk"].rearrange("(co ci) n -> ci co n", ci=P))
        nc.sync.dma_start(WvH[:], d["wv"].rearrange("(co ci) n -> ci co n", ci=P))
        nc.sync.dma_start(identh[:], d["identh"][:])
        nc.sync.dma_start(maskt[:], d["maskc"][:])
        for r in range(1, 4):
            nc.sync.dma_start(xT[:, :, ts(r, 512)], xtv[:, :, ts(r, 512)])
        nc.sync.dma_start(
            WoH[:, :3, :], d["wo"][:3 * P, :].rearrange("(co ci) n -> ci co n", ci=P)
        )
        nc.sync.dma_start(WoH[:64, 3, :], d["wo"][3 * P:, :])
        nc.gpsimd.memset(warm[:], 0.0)
        nc.gpsimd.memset(V[:, :, :, 32:33], 1.0)
        nc.gpsimd.memset(K8[:, 1, :], 0.0)
        for zs in range(4):
            nc.gpsimd.memset(Q8[:, zs, 1, :], 0.0)

        # p-state warmup: the PE ramps 0.65 -> 1.2 -> 2.4 GHz over ~3us of
        # continuous execution; burn that ramp on dummy matmuls while the
        # input DMAs are still in flight so real work runs at full clock
        for wi in range(3):
            wsp = spp.tile([P, 2, 512], F32, tag="sp", name="wsp")
            for sl in range(2):
                nc.tensor.matmul(
                    wsp[:, sl, :], lhsT=warm[:, 0:P], rhs=warm[:],
                    start=True, stop=True, skip_group_check=True,
                )

        # two heads share one av PSUM tile (PSUM pools are bank-granular, so
        # a 528B single-head tile would waste a whole 2KB bank per buffer)
        avh = {"tile": None}

        def emit_av(AO, h, qc, pts):
            # one contiguous accumulation chain per q-subblock j (PSUM banks
            # support a single open matmul accumulation group at a time)
            g, _ = _head_gs(h)
            if h % 2 == 0 or avh["tile"] is None:
                avh["tile"] = avp.tile([P, 2, 4, 33], F32, tag="av", name="av")
            av = avh["tile"]
            hs = h % 2
            for j in range(4):
                for ki in range(4 * qc + j + 1):
                    tl, sub = pts[ki]
                    nc.tensor.matmul(
                        av[:, hs, j, 0:33],
                        lhsT=tl[:, sub, ts(j, P)],
                        rhs=V[:, ki, g, 0:33],
                        start=(ki == 0),
                        stop=(ki == 4 * qc + j),
                        skip_group_check=True,
                    )
            if hs == 1:
                # normalize both heads of the pair in one recip + one TT
                rz = rzp.tile([P, 8, 1], F32, tag="rz", name="rz")
                nc.vector.reciprocal_approx_fast(
                    rz[:], av[:, :, :, 32:33].rearrange("p a b c -> p (a b) c")
                )
                eng_load["dve"] += 80.0 + (2 * 4 * 32 * 1.0417 + 125)
                nc.vector.tensor_tensor(
                    AO[:, :, (h - 1) * D:(h + 1) * D].rearrange(
                        "p j (hh e) -> p hh j e", hh=2
                    ),
                    av[:, :, :, 0:D],
                    rz.rearrange("p (a b) c -> p a b c", a=2).to_broadcast(
                        (P, 2, 4, D)
                    ),
                    MULT,
                )

        def head_block(AO, h, qc):
            # S chunks tiled as triples off the diagonal (fewer, wider exp
            # ops) and pairs on the diagonal (preserves the fully-masked
            # column trim); returns per-chunk (tile, sub-index) handles
            g, s = _head_gs(h)
            qs = qc * 512
            nch = 4 * qc + 4
            sizes = []
            left = nch - 4
            while left > 0:
                if left == 4:
                    sizes += [2, 2]
                    left = 0
                elif left >= 3:
                    sizes.append(3)
                    left -= 3
                else:
                    sizes.append(left)
                    left = 0
            sizes += [2, 2]
            chunk_map = []
            ki = 0
            for n in sizes:
                sp = spp.tile([P, n, 512], F32, tag="sp")
                pt = ptp.tile([P, n, 512], F16, tag="pt")
                for i in range(n):
                    kk = ki + i
                    dg = kk - 4 * qc
                    qoff = 0 if dg < 0 else P * dg
                    nc.tensor.matmul(
                        sp[:, i, qoff:512],
                        lhsT=K8[ts(g, D), :, kk * P:kk * P + P],
                        rhs=Q8[ts(g, D), s, :, qs + qoff:qs + 512],
                        start=True, stop=True, skip_group_check=True,
                        tile_position=(g * D, 0), perf_mode=DR,
                    )
                    chunk_map.append((pt, i))
                qoffE = P * max(0, ki - 4 * qc)
                emit_exp(pt, sp, n, qoffE)
                for i in range(n):
                    dg = ki + i - 4 * qc
                    if dg >= 0:
                        emit_mask(pt[:, i, P * dg:P * dg + P], maskt)
                ki += n
            return chunk_map

        def prep_pieces(tc):
            # Q/K/V projections for round tc, as closures so they can be
            # interleaved into the previous round's head stream.

            def qproj(s):
                def f():
                    Ms = SLOTW[s]
                    qp = ppp.tile([P, 512], F32, tag="pp", name="qp")
                    for cc in range(7):
                        nc.tensor.matmul(
                            qp[0:Ms, :],
                            lhsT=WqH[:, cc, SOFF[s]:SOFF[s] + Ms],
                            rhs=xT[:, cc, ts(tc, 512)],
                            start=(cc == 0), stop=(cc == 6),
                        )
                    emit_copy(Q8[0:Ms, s, 0, ts(tc, 512)], qp[0:Ms, :])
                return f

            def kproj():
                kp = ppp.tile([P, 512], F32, tag="pp", name="kp")
                for cc in range(7):
                    nc.tensor.matmul(
                        kp[:],
                        lhsT=WkH[:, cc, :],
                        rhs=xT[:, cc, ts(tc, 512)],
                        start=(cc == 0), stop=(cc == 6),
                    )
                emit_copy(K8[:, 0, ts(tc, 512)], kp[:])

            def vproj():
                # all four 128-token sub-blocks of this round in one PSUM
                # tile so the copy-out is a single wide op
                vp = ppp.tile([P, 4, P], F32, tag="pp", name="vp")
                for tsub in range(4):
                    kc = 4 * tc + tsub
                    for cc in range(7):
                        nc.tensor.matmul(
                            vp[:, tsub, :],
                            lhsT=xT[:, cc, ts(kc, P)],
                            rhs=WvH[:, cc, :],
                            start=(cc == 0), stop=(cc == 6),
                            skip_group_check=True,
                        )
                emit_copy(
                    V[:, 4 * tc:4 * tc + 4, :, 0:D],
                    vp.rearrange("p t (g e) -> p t g e", g=4),
                )

            return [qproj(s) for s in range(4)] + [kproj, vproj]

        def posts_pieces(tc, AO):
            # AO transpose + out-projection for round tc

            def aotr(c):
                def f():
                    M = P if c < 3 else 64
                    tr = ppp.tile([P, 4, P], F16, tag="pp", name="tr")
                    for j in range(4):
                        nc.tensor.transpose(
                            tr[0:M, j, :], AO[:, j, c * P:c * P + M], identh[:]
                        )
                    emit_copy(
                        AOT[0:M, c, ts(tc, 512)], tr[0:M, :, :], src16=True
                    )
                return f

            def oproj(tsub):
                def f():
                    tg = 4 * tc + tsub
                    ob = obp.tile([P, C], F16, tag="ob")
                    for ncol in range(2):
                        po = ppp.tile([P, 512], F32, tag="pp", name="po")
                        for c in range(4):
                            K = P if c < 3 else 64
                            nc.tensor.matmul(
                                po[:, 0:DH],
                                lhsT=AOT[0:K, c, ts(tg, P)],
                                rhs=WoH[0:K, c, ncol * DH:(ncol + 1) * DH],
                                start=(c == 0), stop=(c == 3),
                                skip_group_check=True,
                            )
                        emit_copy(ob[:, ncol * DH:(ncol + 1) * DH], po[:, 0:DH])
                    nc.sync.dma_start(ov[:, tg, :], ob[:])
                return f

            return [aotr(c) for c in range(4)] + [oproj(t) for t in range(4)]

        for piece in prep_pieces(0):
            piece()
        prevAO = None
        for tc in range(4):
            # pending work interleaved into this round's head stream: next
            # round's projections + previous round's output projection
            pend = []
            if tc < 3:
                pend += prep_pieces(tc + 1)
            if prevAO is not None:
                pend += posts_pieces(tc - 1, prevAO)
            AO = aop.tile([P, 4, DH], F16, tag="AO")
            aotrs3 = oprojs3 = None
            if tc == 3:
                p3 = posts_pieces(3, AO)
                aotrs3, oprojs3 = p3[:4], p3[4:]
            prev_pts = None
            for h in range(HL):
                pts = head_block(AO, h, tc)
                if prev_pts is not None:
                    emit_av(AO, h - 1, tc, prev_pts)
                    if tc == 3 and (h - 1) % 4 == 3:
                        aotrs3[(h - 1) // 4]()
                prev_pts = pts
                for _ in range(2):
                    if pend:
                        pend.pop(0)()
            emit_av(AO, HL - 1, tc, prev_pts)
            for piece in pend:
                piece()
            if tc == 0 and "ao_d" in d:
                nc.sync.dma_start(
                    d["ao_d"].rearrange("p (j e) -> p j e", j=4), AO[:]
                )
            if tc == 3:
                aotrs3[3]()
                for piece in oprojs3:
                    piece()
            prevAO = AO

        if "q8_d" in d:
            nc.sync.dma_start(
                d["q8_d"].rearrange("p (s i t) -> p s i t", s=4, i=2), Q8[:]
            )
            nc.sync.dma_start(d["k8_d"].rearrange("p (i t) -> p i t", i=2), K8[:])
            nc.sync.dma_start(
                d["v_d"].rearrange("p (a b c) -> p a b c", a=16, b=4), V[:]
            )


_NC_CACHE = None


def _build():
    global _NC_CACHE
    if _NC_CACHE is not None:
        return _NC_CACHE
    nc = bacc.Bacc("TRN2", target_bir_lowering=False, debug=False, num_devices=8)
    d = {
        "xt": nc.dram_tensor("xt", (C, T), F16, kind="ExternalInput"),
        "wq": nc.dram_tensor("wq", (C, DH), F16, kind="ExternalInput"),
        "wk": nc.dram_tensor("wk", (C, P), F16, kind="ExternalInput"),
        "wv": nc.dram_tensor("wv", (C, P), F16, kind="ExternalInput"),
        "wo": nc.dram_tensor("wo", (DH, C), F16, kind="ExternalInput"),
        "identh": nc.dram_tensor("identh", (P, P), F16, kind="ExternalInput"),
        "maskc": nc.dram_tensor("maskc", (P, P), F16, kind="ExternalInput"),
        "out": nc.dram_tensor("out", (T, C), F16, kind="ExternalOutput"),
    }
    with tile.TileContext(nc) as tc_:
        _trace(tc_, {k: v[:] for k, v in d.items()})
    nc.compile()
    _NC_CACHE = nc
    return nc


def _in_maps(x, Wq, Wk, Wv, Wo):
    identh = np.eye(P, dtype=np.float16)
    # maskc[p, j] = 0 where q-local j < kpos-local p (strict causal mask),
    # else 1; multiplied into P^T diagonal blocks post-exp
    maskc = np.where(
        np.arange(P)[None, :] < np.arange(P)[:, None], 0.0, 1.0
    ).astype(np.float16)
    maps = []
    for core in range(8):
        b, hf = core // 2, core % 2
        # Wq cols: slot-major [s, g, d] ordering
        qcols = []
        for s in range(4):
            for g in range(4 if s < 2 else 3):
                hloc = g * 4 + s if g < 3 else 12 + s
                H = HEADS_HALF[hf][hloc]
                qcols.extend(range(32 * H, 32 * H + 32))
        # Wk/Wv cols: group-major [g, d]
        kcols = np.concatenate(
            [np.arange(32 * kv, 32 * kv + 32) for kv in KV_HALF[hf]]
        )
        # Wo rows: local-head-major [h, d]
        orows = np.concatenate(
            [np.arange(32 * H, 32 * H + 32) for H in HEADS_HALF[hf]]
        )
        maps.append(
            {
                "xt": np.ascontiguousarray(x[b].T).astype(np.float16),
                "wq": np.ascontiguousarray(Wq[:, qcols] * WSCALE).astype(np.float16),
                "wk": np.ascontiguousarray(Wk[:, kcols] * WSCALE).astype(np.float16),
                "wv": np.ascontiguousarray(Wv[:, kcols]).astype(np.float16),
                "wo": np.ascontiguousarray(Wo[orows, :]).astype(np.float16),
                "identh": identh,
                "maskc": maskc,
            }
        )
    return maps


def run(x, Wq, Wk, Wv, Wo, trace=False):
    nc = _build()
    res = run_bass_kernel_spmd(
        nc, _in_maps(x, Wq, Wk, Wv, Wo), core_ids=list(range(8)), trace=trace
    )
    outs = [r["out"] for r in res.results]
    final = np.empty((4, T, C), np.float32)
    for b in range(4):
        final[b] = outs[2 * b].astype(np.float32) + outs[2 * b + 1].astype(
            np.float32
        )
    return final, res


def kernel(x, Wq, Wk, Wv, Wo):
    x = np.asarray(x, dtype=np.float32)
    out, _ = run(
        x,
        np.asarray(Wq, np.float32),
        np.asarray(Wk, np.float32),
        np.asarray(Wv, np.float32),
        np.asarray(Wo, np.float32),
    )
    return out


# revision 43
# speedup vs baseline: 1.1746x; 1.1746x over previous
"""Causal GQA self-attention on 8 Trainium2 NeuronCores (fp16 pipeline).

Sharding: data-parallel over batch (4) x tensor-parallel over heads (2 halves
of 14 heads each, KV heads replicated per GQA group). Each core computes its
heads' partial contribution through the row-parallel out-projection; the host
sums the two fp16 partials per batch element in fp32.

Per-core local structure: 4 local kv groups g (sizes 4,4,4,2 heads), local
head h -> (g = h//4, s = h%4 slot). All tensors fp16 except PSUM (f32).

Layouts (SBUF [128 partitions, free...]):
  xT  [128, 7, 2048]   x^T (C on partitions), DMA'd directly (host pre-transposes)
  QT  [128, 4, 2048]   Q^T: head (g,s) at partitions 32g..32g+32, slot s
  KT  [128, 2048]      K^T: group g at partitions 32g..32g+32
  V   [128, 16, 4, 33] V rows (kpos%128 on partitions), col 32 = ones (Z)
  AO  [128, 4, 448]    attn out rows (q%128 on partitions) per 512-q round
  AOT [128, 4, 2048]   attn out transposed (head dims on partitions)

Pipeline: 4 rounds of 512 tokens; per round: Q/K/V projection, then per head:
S^T = K^T.T @ Q^T per 128-kpos chunk (diagonal chunks get a mask preloaded
into PSUM via an extra matmul, then accumulate), exp greedily load-balanced
across ScalarE (exact), DVE and Pool/GPSIMD (Schraudolph fast-exp: bit-trick
y = s*A+B -> int16 -> reinterpret as fp16), then AV flipped: out[q,d] with
P^T chunk as stationary operand and [V | 1] as 33-wide moving operand so the
softmax denominator Z rides along as column 32. Normalize batched per
(head, round), transpose AO via PE, row-parallel out-projection, fp16
partial out. All elementwise work (exp, PSUM->SBUF copies, normalize) is
assigned per-op to the least-loaded of {Act, DVE, Pool} by a static greedy
cost model.
"""

import sys

sys.path.insert(0, "/opt/trn_rl_repo")

import numpy as np

import concourse.bass as bass
import concourse.mybir as mybir
import concourse.tile as tile
from concourse import bacc
from concourse.bass import ts
from concourse.bass_utils import run_bass_kernel_spmd

F32 = mybir.dt.float32
F16 = mybir.dt.float16
F8 = mybir.dt.float8e4
I16 = mybir.dt.int16
EXP = mybir.ActivationFunctionType.Exp
COPY = mybir.ActivationFunctionType.Copy
DR = mybir.MatmulPerfMode.DoubleRow
MULT = mybir.AluOpType.mult
ADD = mybir.AluOpType.add
P = 128
T, C = 2048, 896
D = 32
HL = 14          # local heads per core
DH = HL * D      # 448
SCALE = 1.0 / float(np.sqrt(D))
# Wq/Wk are pre-scaled x16 on the host so q,k land in fp8e4m3's sweet spot;
# scores come out x256, compensated in the exp scale / Schraudolph slope.
WSCALE = 16.0
SSCALE = SCALE / (WSCALE * WSCALE)
MASKVAL = -180.0 * WSCALE * WSCALE
# Schraudolph fast-exp consts (fp16 bit trick): y = s*A + B as int16
A_S = SSCALE * 1024.0 / float(np.log(2.0))
B_S = 15.0 * 1024.0 - 0.043 * 1024.0

SOFF = [0, 128, 256, 352]   # Wq col offset per slot
SLOTW = [128, 128, 96, 96]  # slot widths (s>=2 lack group 3)

HEADS_HALF = [
    list(range(0, 12)) + [24, 25],
    list(range(12, 24)) + [26, 27],
]
KV_HALF = [[0, 1, 2, 6], [3, 4, 5, 6]]


def _head_gs(h):
    return (h // 4, h % 4) if h < 12 else (3, h - 12)


def _trace(tc_, d):
    nc = tc_.nc

    # greedy elementwise load balancing across Act / DVE / Pool
    eng_load = {"act": 0.0, "dve": 0.0, "pool": 0.0}

    def pick(costs):
        e = min(costs, key=lambda k: eng_load[k] + costs[k])
        eng_load[e] += costs[e]
        return e

    def copy_cost(w, src16=False):
        # f32-PSUM (or f16-PSUM when src16) -> SBUF f16 copy costs
        # (GPSIMD/Pool cannot access PSUM on TRN2, so only Act/DVE here)
        dvec = (0.52 if src16 else 1.0417) * w + 125
        return {"act": 0.833 * w + 185, "dve": dvec}

    def emit_copy(dst, src, src16=False, scale=None):
        e = pick(copy_cost(src.free_size(), src16))
        if e == "act":
            nc.scalar.activation(dst, src, COPY, scale=1.0 if scale is None else scale)
        else:
            if scale is None:
                nc.vector.tensor_copy(dst, src)
            else:
                nc.vector.tensor_scalar_mul(dst, src, scale)

    def emit_exp(pt, sp, n, qoffE):
        w = n * (512 - qoffE)
        e = pick({"act": 0.833 * w + 185,
                  "dve": 1.0417 * w + 125})
        if e == "act":
            nc.scalar.activation(
                pt[:, :, qoffE:512], sp[:, :, qoffE:512], EXP, scale=SSCALE
            )
        else:
            nc.vector.tensor_scalar(
                pt[:, :, qoffE:512].bitcast(I16),
                sp[:, :, qoffE:512], A_S, B_S, MULT, ADD,
            )

    def emit_mask(ap, maskt):
        # zero the strictly-upper (future) triangle of a diagonal 128x128
        # P^T block post-exp; SBUF-only, so the otherwise-idle Pool engine
        # can absorb most of these
        e = pick({"dve": 0.52 * 128 + 60, "pool": 95 + 1.984 * 128})
        tt = nc.vector.tensor_tensor if e == "dve" else nc.gpsimd.tensor_tensor
        tt(ap, ap, maskt[:], MULT)

    with tc_.tile_pool(name="const", bufs=1) as const, \
         tc_.tile_pool(name="persist", bufs=1) as persist, \
         tc_.tile_pool(name="aop", bufs=2) as aop, \
         tc_.tile_pool(name="ptp", bufs=16) as ptp, \
         tc_.tile_pool(name="rzp", bufs=2) as rzp, \
         tc_.tile_pool(name="obp", bufs=2) as obp, \
         tc_.tile_pool(name="spp", bufs=3, space="PSUM") as spp, \
         tc_.tile_pool(name="ppp", bufs=1, space="PSUM") as ppp, \
         tc_.tile_pool(name="avp", bufs=1, space="PSUM") as avp:

        identh = const.tile([P, P], F16)
        maskt = const.tile([P, P], F16)
        warm = const.tile([P, 512], F16)

        xT = persist.tile([P, 7, T], F16, tag="xT")
        # Q8/K8 hold x16-scaled q,k in fp8e4m3 for the DoubleRow S matmul.
        # The second k-tile (index 1) is zeroed once and never written again:
        # DoubleRow contracts over 2 k-tiles, and padding the second with
        # zeros gives a plain 32-deep contraction at 0.5 cycles/col.
        Q8 = persist.tile([P, 4, 2, T], F8, tag="Q8")
        K8 = persist.tile([P, 2, T], F8, tag="K8")
        V = persist.tile([P, 16, 4, 33], F16, tag="V")
        AOT = persist.tile([P, 4, T], F16, tag="AOT")
        WqH = persist.tile([P, 7, DH], F16, tag="WqH")
        WkH = persist.tile([P, 7, P], F16, tag="WkH")
        WvH = persist.tile([P, 7, P], F16, tag="WvH")
        WoH = persist.tile([P, 4, C], F16, tag="WoH")

        xtv = d["xt"].rearrange("(co ci) t -> ci co t", ci=P)
        ov = d["out"].rearrange("(to ti) c -> ti to c", ti=P)

        # input DMAs: round-0 x^T first, then QKV weights, rest of x^T, Wo
        nc.sync.dma_start(xT[:, :, 0:512], xtv[:, :, 0:512])
        nc.sync.dma_start(WqH[:], d["wq"].rearrange("(co ci) n -> ci co n", ci=P))
        nc.sync.dma_start(WkH[:], d["wk"].rearrange("(co ci) n -> ci co n", ci=P))
        nc.sync.dma_start(WvH[:], d["wv"].rearrange("(co ci) n -> ci co n", ci=P))
        nc.sync.dma_start(identh[:], d["identh"][:])
        nc.sync.dma_start(maskt[:], d["maskc"][:])
        for r in range(1, 4):
            nc.sync.dma_start(xT[:, :, ts(r, 512)], xtv[:, :, ts(r, 512)])
        nc.sync.dma_start(
            WoH[:, :3, :], d["wo"][:3 * P, :].rearrange("(co ci) n -> ci co n", ci=P)
        )
        nc.sync.dma_start(WoH[:64, 3, :], d["wo"][3 * P:, :])
        nc.gpsimd.memset(warm[:], 0.0)
        nc.gpsimd.memset(V[:, :, :, 32:33], 1.0)
        nc.gpsimd.memset(K8[:, 1, :], 0.0)
        for zs in range(4):
            nc.gpsimd.memset(Q8[:, zs, 1, :], 0.0)

        # p-state warmup: the PE ramps 0.65 -> 1.2 -> 2.4 GHz over ~3us of
        # continuous execution; burn that ramp on dummy matmuls while the
        # input DMAs are still in flight so real work runs at full clock
        for wi in range(3):
            wsp = spp.tile([P, 2, 512], F32, tag="sp", name="wsp")
            for sl in range(2):
                nc.tensor.matmul(
                    wsp[:, sl, :], lhsT=warm[:, 0:P], rhs=warm[:],
                    start=True, stop=True, skip_group_check=True,
                )

        # two heads share one av PSUM tile (PSUM pools are bank-granular, so
        # a 528B single-head tile would waste a whole 2KB bank per buffer)
        avh = {"tile": None}

        def emit_av(AO, h, qc, pts):
            # one contiguous accumulation chain per q-subblock j (PSUM banks
            # support a single open matmul accumulation group at a time)
            g, _ = _head_gs(h)
            if h % 2 == 0 or avh["tile"] is None:
                avh["tile"] = avp.tile([P, 2, 4, 33], F32, tag="av", name="av")
            av = avh["tile"]
            hs = h % 2
            for j in range(4):
                for ki in range(4 * qc + j + 1):
                    tl, sub = pts[ki]
                    nc.tensor.matmul(
                        av[:, hs, j, 0:33],
                        lhsT=tl[:, sub, ts(j, P)],
                        rhs=V[:, ki, g, 0:33],
                        start=(ki == 0),
                        stop=(ki == 4 * qc + j),
                        skip_group_check=True,
                    )
            if hs == 1:
                # normalize both heads of the pair in one recip + one TT
                rz = rzp.tile([P, 8, 1], F32, tag="rz", name="rz")
                nc.vector.reciprocal_approx_fast(
                    rz[:], av[:, :, :, 32:33].rearrange("p a b c -> p (a b) c")
                )
                eng_load["dve"] += 80.0 + (2 * 4 * 32 * 1.0417 + 125)
                nc.vector.tensor_tensor(
                    AO[:, :, (h - 1) * D:(h + 1) * D].rearrange(
                        "p j (hh e) -> p hh j e", hh=2
                    ),
                    av[:, :, :, 0:D],
                    rz.rearrange("p (a b) c -> p a b c", a=2).to_broadcast(
                        (P, 2, 4, D)
                    ),
                    MULT,
                )

        def head_block(AO, h, qc):
            # S chunks tiled as triples off the diagonal (fewer, wider exp
            # ops) and pairs on the diagonal (preserves the fully-masked
            # column trim); returns per-chunk (tile, sub-index) handles
            g, s = _head_gs(h)
            qs = qc * 512
            nch = 4 * qc + 4
            sizes = [2] * (nch // 2)
            chunk_map = []
            ki = 0
            for n in sizes:
                sp = spp.tile([P, n, 512], F32, tag="sp")
                pt = ptp.tile([P, n, 512], F16, tag="pt")
                for i in range(n):
                    kk = ki + i
                    dg = kk - 4 * qc
                    qoff = 0 if dg < 0 else P * dg
                    nc.tensor.matmul(
                        sp[:, i, qoff:512],
                        lhsT=K8[ts(g, D), :, kk * P:kk * P + P],
                        rhs=Q8[ts(g, D), s, :, qs + qoff:qs + 512],
                        start=True, stop=True, skip_group_check=True,
                        tile_position=(g * D, 0), perf_mode=DR,
                    )
                    chunk_map.append((pt, i))
                qoffE = P * max(0, ki - 4 * qc)
                emit_exp(pt, sp, n, qoffE)
                for i in range(n):
                    dg = ki + i - 4 * qc
                    if dg >= 0:
                        emit_mask(pt[:, i, P * dg:P * dg + P], maskt)
                ki += n
            return chunk_map

        def prep_pieces(tc):
            # Q/K/V projections for round tc, as closures so they can be
            # interleaved into the previous round's head stream.

            def qproj(s):
                def f():
                    Ms = SLOTW[s]
                    qp = ppp.tile([P, 512], F32, tag="pp", name="qp")
                    for cc in range(7):
                        nc.tensor.matmul(
                            qp[0:Ms, :],
                            lhsT=WqH[:, cc, SOFF[s]:SOFF[s] + Ms],
                            rhs=xT[:, cc, ts(tc, 512)],
                            start=(cc == 0), stop=(cc == 6),
                        )
                    emit_copy(Q8[0:Ms, s, 0, ts(tc, 512)], qp[0:Ms, :])
                return f

            def kproj():
                kp = ppp.tile([P, 512], F32, tag="pp", name="kp")
                for cc in range(7):
                    nc.tensor.matmul(
                        kp[:],
                        lhsT=WkH[:, cc, :],
                        rhs=xT[:, cc, ts(tc, 512)],
                        start=(cc == 0), stop=(cc == 6),
                    )
                emit_copy(K8[:, 0, ts(tc, 512)], kp[:])

            def vproj():
                # all four 128-token sub-blocks of this round in one PSUM
                # tile so the copy-out is a single wide op
                vp = ppp.tile([P, 4, P], F32, tag="pp", name="vp")
                for tsub in range(4):
                    kc = 4 * tc + tsub
                    for cc in range(7):
                        nc.tensor.matmul(
                            vp[:, tsub, :],
                            lhsT=xT[:, cc, ts(kc, P)],
                            rhs=WvH[:, cc, :],
                            start=(cc == 0), stop=(cc == 6),
                            skip_group_check=True,
                        )
                emit_copy(
                    V[:, 4 * tc:4 * tc + 4, :, 0:D],
                    vp.rearrange("p t (g e) -> p t g e", g=4),
                )

            return [qproj(s) for s in range(4)] + [kproj, vproj]

        def posts_pieces(tc, AO):
            # AO transpose + out-projection for round tc

            def aotr(c):
                def f():
                    M = P if c < 3 else 64
                    tr = ppp.tile([P, 4, P], F16, tag="pp", name="tr")
                    for j in range(4):
                        nc.tensor.transpose(
                            tr[0:M, j, :], AO[:, j, c * P:c * P + M], identh[:]
                        )
                    emit_copy(
                        AOT[0:M, c, ts(tc, 512)], tr[0:M, :, :], src16=True
                    )
                return f

            def oproj(tsub):
                def f():
                    tg = 4 * tc + tsub
                    ob = obp.tile([P, C], F16, tag="ob")
                    for ncol in range(2):
                        po = ppp.tile([P, 512], F32, tag="pp", name="po")
                        for c in range(4):
                            K = P if c < 3 else 64
                            nc.tensor.matmul(
                                po[:, 0:DH],
                                lhsT=AOT[0:K, c, ts(tg, P)],
                                rhs=WoH[0:K, c, ncol * DH:(ncol + 1) * DH],
                                start=(c == 0), stop=(c == 3),
                                skip_group_check=True,
                            )
                        emit_copy(ob[:, ncol * DH:(ncol + 1) * DH], po[:, 0:DH])
                    nc.sync.dma_start(ov[:, tg, :], ob[:])
                return f

            return [aotr(c) for c in range(4)] + [oproj(t) for t in range(4)]

        for piece in prep_pieces(0):
            piece()
        prevAO = None
        for tc in range(4):
            # pending work interleaved into this round's head stream: next
            # round's projections + previous round's output projection
            pend = []
            if tc < 3:
                pend += prep_pieces(tc + 1)
            if prevAO is not None:
                pend += posts_pieces(tc - 1, prevAO)
            AO = aop.tile([P, 4, DH], F16, tag="AO")
            aotrs3 = oprojs3 = None
            if tc == 3:
                p3 = posts_pieces(3, AO)
                aotrs3, oprojs3 = p3[:4], p3[4:]
            prev_pts = None
            for h in range(HL):
                pts = head_block(AO, h, tc)
                if prev_pts is not None:
                    emit_av(AO, h - 1, tc, prev_pts)
                    if tc == 3 and (h - 1) % 4 == 3:
                        aotrs3[(h - 1) // 4]()
                prev_pts = pts
                for _ in range(2):
                    if pend:
                        pend.pop(0)()
            emit_av(AO, HL - 1, tc, prev_pts)
            for piece in pend:
                piece()
            if tc == 0 and "ao_d" in d:
                nc.sync.dma_start(
                    d["ao_d"].rearrange("p (j e) -> p j e", j=4), AO[:]
                )
            if tc == 3:
                aotrs3[3]()
                for piece in oprojs3:
                    piece()
            prevAO = AO

        if "q8_d" in d:
            nc.sync.dma_start(
                d["q8_d"].rearrange("p (s i t) -> p s i t", s=4, i=2), Q8[:]
            )
            nc.sync.dma_start(d["k8_d"].rearrange("p (i t) -> p i t", i=2), K8[:])
            nc.sync.dma_start(
                d["v_d"].rearrange("p (a b c) -> p a b c", a=16, b=4), V[:]
            )


_NC_CACHE = None


def _build():
    global _NC_CACHE
    if _NC_CACHE is not None:
        return _NC_CACHE
    nc = bacc.Bacc("TRN2", target_bir_lowering=False, debug=False, num_devices=8)
    d = {
        "xt": nc.dram_tensor("xt", (C, T), F16, kind="ExternalInput"),
        "wq": nc.dram_tensor("wq", (C, DH), F16, kind="ExternalInput"),
        "wk": nc.dram_tensor("wk", (C, P), F16, kind="ExternalInput"),
        "wv": nc.dram_tensor("wv", (C, P), F16, kind="ExternalInput"),
        "wo": nc.dram_tensor("wo", (DH, C), F16, kind="ExternalInput"),
        "identh": nc.dram_tensor("identh", (P, P), F16, kind="ExternalInput"),
        "maskc": nc.dram_tensor("maskc", (P, P), F16, kind="ExternalInput"),
        "out": nc.dram_tensor("out", (T, C), F16, kind="ExternalOutput"),
    }
    with tile.TileContext(nc) as tc_:
        _trace(tc_, {k: v[:] for k, v in d.items()})
    nc.compile()
    _NC_CACHE = nc
    return nc


def _in_maps(x, Wq, Wk, Wv, Wo):
    identh = np.eye(P, dtype=np.float16)
    # maskc[p, j] = 0 where q-local j < kpos-local p (strict causal mask),
    # else 1; multiplied into P^T diagonal blocks post-exp
    maskc = np.where(
        np.arange(P)[None, :] < np.arange(P)[:, None], 0.0, 1.0
    ).astype(np.float16)
    maps = []
    for core in range(8):
        b, hf = core // 2, core % 2
        # Wq cols: slot-major [s, g, d] ordering
        qcols = []
        for s in range(4):
            for g in range(4 if s < 2 else 3):
                hloc = g * 4 + s if g < 3 else 12 + s
                H = HEADS_HALF[hf][hloc]
                qcols.extend(range(32 * H, 32 * H + 32))
        # Wk/Wv cols: group-major [g, d]
        kcols = np.concatenate(
            [np.arange(32 * kv, 32 * kv + 32) for kv in KV_HALF[hf]]
        )
        # Wo rows: local-head-major [h, d]
        orows = np.concatenate(
            [np.arange(32 * H, 32 * H + 32) for H in HEADS_HALF[hf]]
        )
        maps.append(
            {
                "xt": np.ascontiguousarray(x[b].T).astype(np.float16),
                "wq": np.ascontiguousarray(Wq[:, qcols] * WSCALE).astype(np.float16),
                "wk": np.ascontiguousarray(Wk[:, kcols] * WSCALE).astype(np.float16),
                "wv": np.ascontiguousarray(Wv[:, kcols]).astype(np.float16),
                "wo": np.ascontiguousarray(Wo[orows, :]).astype(np.float16),
                "identh": identh,
                "maskc": maskc,
            }
        )
    return maps


def run(x, Wq, Wk, Wv, Wo, trace=False):
    nc = _build()
    res = run_bass_kernel_spmd(
        nc, _in_maps(x, Wq, Wk, Wv, Wo), core_ids=list(range(8)), trace=trace
    )
    outs = [r["out"] for r in res.results]
    final = np.empty((4, T, C), np.float32)
    for b in range(4):
        final[b] = outs[2 * b].astype(np.float32) + outs[2 * b + 1].astype(
            np.float32
        )
    return final, res


def kernel(x, Wq, Wk, Wv, Wo):
    x = np.asarray(x, dtype=np.float32)
    out, _ = run(
        x,
        np.asarray(Wq, np.float32),
        np.asarray(Wk, np.float32),
        np.asarray(Wv, np.float32),
        np.asarray(Wo, np.float32),
    )
    return out


# revision 44
# speedup vs baseline: 1.1927x; 1.0154x over previous
"""Causal GQA self-attention on 8 Trainium2 NeuronCores (fp16 pipeline).

Sharding: data-parallel over batch (4) x tensor-parallel over heads (2 halves
of 14 heads each, KV heads replicated per GQA group). Each core computes its
heads' partial contribution through the row-parallel out-projection; the host
sums the two fp16 partials per batch element in fp32.

Per-core local structure: 4 local kv groups g (sizes 4,4,4,2 heads), local
head h -> (g = h//4, s = h%4 slot). All tensors fp16 except PSUM (f32).

Layouts (SBUF [128 partitions, free...]):
  xT  [128, 7, 2048]   x^T (C on partitions), DMA'd directly (host pre-transposes)
  QT  [128, 4, 2048]   Q^T: head (g,s) at partitions 32g..32g+32, slot s
  KT  [128, 2048]      K^T: group g at partitions 32g..32g+32
  V   [128, 16, 4, 33] V rows (kpos%128 on partitions), col 32 = ones (Z)
  AO  [128, 4, 448]    attn out rows (q%128 on partitions) per 512-q round
  AOT [128, 4, 2048]   attn out transposed (head dims on partitions)

Pipeline: 4 rounds of 512 tokens; per round: Q/K/V projection, then per head:
S^T = K^T.T @ Q^T per 128-kpos chunk (diagonal chunks get a mask preloaded
into PSUM via an extra matmul, then accumulate), exp greedily load-balanced
across ScalarE (exact), DVE and Pool/GPSIMD (Schraudolph fast-exp: bit-trick
y = s*A+B -> int16 -> reinterpret as fp16), then AV flipped: out[q,d] with
P^T chunk as stationary operand and [V | 1] as 33-wide moving operand so the
softmax denominator Z rides along as column 32. Normalize batched per
(head, round), transpose AO via PE, row-parallel out-projection, fp16
partial out. All elementwise work (exp, PSUM->SBUF copies, normalize) is
assigned per-op to the least-loaded of {Act, DVE, Pool} by a static greedy
cost model.
"""

import sys

sys.path.insert(0, "/opt/trn_rl_repo")

import numpy as np

import concourse.bass as bass
import concourse.mybir as mybir
import concourse.tile as tile
from concourse import bacc
from concourse.bass import ts
from concourse.bass_utils import run_bass_kernel_spmd

F32 = mybir.dt.float32
F16 = mybir.dt.float16
F8 = mybir.dt.float8e4
I16 = mybir.dt.int16
EXP = mybir.ActivationFunctionType.Exp
COPY = mybir.ActivationFunctionType.Copy
DR = mybir.MatmulPerfMode.DoubleRow
MULT = mybir.AluOpType.mult
ADD = mybir.AluOpType.add
P = 128
T, C = 2048, 896
D = 32
HL = 14          # local heads per core
DH = HL * D      # 448
SCALE = 1.0 / float(np.sqrt(D))
# Wq/Wk are pre-scaled x16 on the host so q,k land in fp8e4m3's sweet spot;
# scores come out x256, compensated in the exp scale / Schraudolph slope.
WSCALE = 16.0
SSCALE = SCALE / (WSCALE * WSCALE)
MASKVAL = -180.0 * WSCALE * WSCALE
# Schraudolph fast-exp consts (fp16 bit trick): y = s*A + B as int16
A_S = SSCALE * 1024.0 / float(np.log(2.0))
B_S = 15.0 * 1024.0 - 0.043 * 1024.0

SOFF = [0, 128, 256, 352]   # Wq col offset per slot
SLOTW = [128, 128, 96, 96]  # slot widths (s>=2 lack group 3)

HEADS_HALF = [
    list(range(0, 12)) + [24, 25],
    list(range(12, 24)) + [26, 27],
]
KV_HALF = [[0, 1, 2, 6], [3, 4, 5, 6]]


def _head_gs(h):
    return (h // 4, h % 4) if h < 12 else (3, h - 12)


def _trace(tc_, d):
    nc = tc_.nc

    # greedy elementwise load balancing across Act / DVE / Pool
    eng_load = {"act": 0.0, "dve": 0.0, "pool": 0.0}

    def pick(costs):
        e = min(costs, key=lambda k: eng_load[k] + costs[k])
        eng_load[e] += costs[e]
        return e

    def copy_cost(w, src16=False):
        # f32-PSUM (or f16-PSUM when src16) -> SBUF f16 copy costs
        # (GPSIMD/Pool cannot access PSUM on TRN2, so only Act/DVE here)
        dvec = (0.52 if src16 else 1.0417) * w + 125
        return {"act": 0.833 * w + 185, "dve": dvec}

    def emit_copy(dst, src, src16=False, scale=None):
        e = pick(copy_cost(src.free_size(), src16))
        if e == "act":
            nc.scalar.activation(dst, src, COPY, scale=1.0 if scale is None else scale)
        else:
            if scale is None:
                nc.vector.tensor_copy(dst, src)
            else:
                nc.vector.tensor_scalar_mul(dst, src, scale)

    def emit_exp(pt, sp, n, qoffE):
        w = n * (512 - qoffE)
        e = pick({"act": 0.833 * w + 185,
                  "dve": 1.0417 * w + 125})
        if e == "act":
            nc.scalar.activation(
                pt[:, :, qoffE:512], sp[:, :, qoffE:512], EXP, scale=SSCALE
            )
        else:
            nc.vector.tensor_scalar(
                pt[:, :, qoffE:512].bitcast(I16),
                sp[:, :, qoffE:512], A_S, B_S, MULT, ADD,
            )

    def emit_mask(ap, maskt):
        # zero the strictly-upper (future) triangle of a diagonal 128x128
        # P^T block post-exp; SBUF-only, so the otherwise-idle Pool engine
        # can absorb most of these
        e = pick({"dve": 0.52 * 128 + 60, "pool": 95 + 1.984 * 128})
        tt = nc.vector.tensor_tensor if e == "dve" else nc.gpsimd.tensor_tensor
        tt(ap, ap, maskt[:], MULT)

    with tc_.tile_pool(name="const", bufs=1) as const, \
         tc_.tile_pool(name="persist", bufs=1) as persist, \
         tc_.tile_pool(name="aop", bufs=2) as aop, \
         tc_.tile_pool(name="ptp", bufs=16) as ptp, \
         tc_.tile_pool(name="rzp", bufs=2) as rzp, \
         tc_.tile_pool(name="obp", bufs=2) as obp, \
         tc_.tile_pool(name="spp", bufs=3, space="PSUM") as spp, \
         tc_.tile_pool(name="ppp", bufs=1, space="PSUM") as ppp, \
         tc_.tile_pool(name="avp", bufs=1, space="PSUM") as avp:

        identh = const.tile([P, P], F16)
        maskt = const.tile([P, P], F16)
        warm = const.tile([P, 512], F16)

        xT = persist.tile([P, 7, T], F16, tag="xT")
        # Q8/K8 hold x16-scaled q,k in fp8e4m3 for the DoubleRow S matmul.
        # The second k-tile (index 1) is zeroed once and never written again:
        # DoubleRow contracts over 2 k-tiles, and padding the second with
        # zeros gives a plain 32-deep contraction at 0.5 cycles/col.
        Q8 = persist.tile([P, 4, 2, T], F8, tag="Q8")
        K8 = persist.tile([P, 2, T], F8, tag="K8")
        V = persist.tile([P, 16, 4, 33], F16, tag="V")
        AOT = persist.tile([P, 4, T], F16, tag="AOT")
        WqH = persist.tile([P, 7, DH], F16, tag="WqH")
        WkH = persist.tile([P, 7, P], F16, tag="WkH")
        WvH = persist.tile([P, 7, P], F16, tag="WvH")
        WoH = persist.tile([P, 4, C], F16, tag="WoH")

        xtv = d["xt"].rearrange("(co ci) t -> ci co t", ci=P)
        ov = d["out"].rearrange("(to ti) c -> ti to c", ti=P)

        # input DMAs: round-0 x^T first, then QKV weights, rest of x^T, Wo
        nc.sync.dma_start(xT[:, :, 0:512], xtv[:, :, 0:512])
        nc.sync.dma_start(WqH[:], d["wq"].rearrange("(co ci) n -> ci co n", ci=P))
        nc.sync.dma_start(WkH[:], d["wk"].rearrange("(co ci) n -> ci co n", ci=P))
        nc.sync.dma_start(WvH[:], d["wv"].rearrange("(co ci) n -> ci co n", ci=P))
        nc.sync.dma_start(identh[:], d["identh"][:])
        nc.sync.dma_start(maskt[:], d["maskc"][:])
        for r in range(1, 4):
            nc.sync.dma_start(xT[:, :, ts(r, 512)], xtv[:, :, ts(r, 512)])
        nc.sync.dma_start(
            WoH[:, :3, :], d["wo"][:3 * P, :].rearrange("(co ci) n -> ci co n", ci=P)
        )
        nc.sync.dma_start(WoH[:64, 3, :], d["wo"][3 * P:, :])
        nc.gpsimd.memset(warm[:], 0.0)
        nc.gpsimd.memset(V[:, :, :, 32:33], 1.0)
        nc.gpsimd.memset(K8[:, 1, :], 0.0)
        for zs in range(4):
            nc.gpsimd.memset(Q8[:, zs, 1, :], 0.0)

        # p-state warmup: the PE ramps 0.65 -> 1.2 -> 2.4 GHz over ~3us of
        # continuous execution; burn that ramp on dummy matmuls while the
        # input DMAs are still in flight so real work runs at full clock
        for wi in range(3):
            wsp = spp.tile([P, 2, 512], F32, tag="sp", name="wsp")
            for sl in range(2):
                nc.tensor.matmul(
                    wsp[:, sl, :], lhsT=warm[:, 0:P], rhs=warm[:],
                    start=True, stop=True, skip_group_check=True,
                )

        # two heads share one av PSUM tile (PSUM pools are bank-granular, so
        # a 528B single-head tile would waste a whole 2KB bank per buffer)
        avh = {"tile": None}

        def emit_av(AO, h, qc, pts):
            # one contiguous accumulation chain per q-subblock j (PSUM banks
            # support a single open matmul accumulation group at a time)
            g, _ = _head_gs(h)
            if h % 2 == 0 or avh["tile"] is None:
                avh["tile"] = avp.tile([P, 2, 4, 33], F32, tag="av", name="av")
            av = avh["tile"]
            hs = h % 2
            for j in range(4):
                for ki in range(4 * qc + j + 1):
                    tl, sub = pts[ki]
                    nc.tensor.matmul(
                        av[:, hs, j, 0:33],
                        lhsT=tl[:, sub, ts(j, P)],
                        rhs=V[:, ki, g, 0:33],
                        start=(ki == 0),
                        stop=(ki == 4 * qc + j),
                        skip_group_check=True,
                    )
            rz = rzp.tile([P, 4, 1], F32, tag="rz", name="rz")
            nc.vector.reciprocal_approx_fast(rz[:], av[:, hs, :, 32:33])
            eng_load["dve"] += 75.0 + (4 * 32 * 1.0417 + 125)
            nc.vector.tensor_tensor(
                AO[:, :, h * D:(h + 1) * D],
                av[:, hs, :, 0:D],
                rz.to_broadcast((P, 4, D)),
                MULT,
            )

        def head_block(AO, h, qc):
            # S chunks tiled as triples off the diagonal (fewer, wider exp
            # ops) and pairs on the diagonal (preserves the fully-masked
            # column trim); returns per-chunk (tile, sub-index) handles
            g, s = _head_gs(h)
            qs = qc * 512
            nch = 4 * qc + 4
            sizes = [2] * (nch // 2)
            chunk_map = []
            ki = 0
            for n in sizes:
                sp = spp.tile([P, n, 512], F32, tag="sp")
                pt = ptp.tile([P, n, 512], F16, tag="pt")
                for i in range(n):
                    kk = ki + i
                    dg = kk - 4 * qc
                    qoff = 0 if dg < 0 else P * dg
                    nc.tensor.matmul(
                        sp[:, i, qoff:512],
                        lhsT=K8[ts(g, D), :, kk * P:kk * P + P],
                        rhs=Q8[ts(g, D), s, :, qs + qoff:qs + 512],
                        start=True, stop=True, skip_group_check=True,
                        tile_position=(g * D, 0), perf_mode=DR,
                    )
                    chunk_map.append((pt, i))
                qoffE = P * max(0, ki - 4 * qc)
                emit_exp(pt, sp, n, qoffE)
                for i in range(n):
                    dg = ki + i - 4 * qc
                    if dg >= 0:
                        emit_mask(pt[:, i, P * dg:P * dg + P], maskt)
                ki += n
            return chunk_map

        def prep_pieces(tc):
            # Q/K/V projections for round tc, as closures so they can be
            # interleaved into the previous round's head stream.

            def qproj(s):
                def f():
                    Ms = SLOTW[s]
                    qp = ppp.tile([P, 512], F32, tag="pp", name="qp")
                    for cc in range(7):
                        nc.tensor.matmul(
                            qp[0:Ms, :],
                            lhsT=WqH[:, cc, SOFF[s]:SOFF[s] + Ms],
                            rhs=xT[:, cc, ts(tc, 512)],
                            start=(cc == 0), stop=(cc == 6),
                        )
                    emit_copy(Q8[0:Ms, s, 0, ts(tc, 512)], qp[0:Ms, :])
                return f

            def kproj():
                kp = ppp.tile([P, 512], F32, tag="pp", name="kp")
                for cc in range(7):
                    nc.tensor.matmul(
                        kp[:],
                        lhsT=WkH[:, cc, :],
                        rhs=xT[:, cc, ts(tc, 512)],
                        start=(cc == 0), stop=(cc == 6),
                    )
                emit_copy(K8[:, 0, ts(tc, 512)], kp[:])

            def vproj():
                # all four 128-token sub-blocks of this round in one PSUM
                # tile so the copy-out is a single wide op
                vp = ppp.tile([P, 4, P], F32, tag="pp", name="vp")
                for tsub in range(4):
                    kc = 4 * tc + tsub
                    for cc in range(7):
                        nc.tensor.matmul(
                            vp[:, tsub, :],
                            lhsT=xT[:, cc, ts(kc, P)],
                            rhs=WvH[:, cc, :],
                            start=(cc == 0), stop=(cc == 6),
                            skip_group_check=True,
                        )
                emit_copy(
                    V[:, 4 * tc:4 * tc + 4, :, 0:D],
                    vp.rearrange("p t (g e) -> p t g e", g=4),
                )

            return [qproj(s) for s in range(4)] + [kproj, vproj]

        def posts_pieces(tc, AO):
            # AO transpose + out-projection for round tc

            def aotr(c):
                def f():
                    M = P if c < 3 else 64
                    tr = ppp.tile([P, 4, P], F16, tag="pp", name="tr")
                    for j in range(4):
                        nc.tensor.transpose(
                            tr[0:M, j, :], AO[:, j, c * P:c * P + M], identh[:]
                        )
                    emit_copy(
                        AOT[0:M, c, ts(tc, 512)], tr[0:M, :, :], src16=True
                    )
                return f

            def oproj(tsub):
                def f():
                    tg = 4 * tc + tsub
                    ob = obp.tile([P, C], F16, tag="ob")
                    for ncol in range(2):
                        po = ppp.tile([P, 512], F32, tag="pp", name="po")
                        for c in range(4):
                            K = P if c < 3 else 64
                            nc.tensor.matmul(
                                po[:, 0:DH],
                                lhsT=AOT[0:K, c, ts(tg, P)],
                                rhs=WoH[0:K, c, ncol * DH:(ncol + 1) * DH],
                                start=(c == 0), stop=(c == 3),
                                skip_group_check=True,
                            )
                        emit_copy(ob[:, ncol * DH:(ncol + 1) * DH], po[:, 0:DH])
                    nc.sync.dma_start(ov[:, tg, :], ob[:])
                return f

            return [aotr(c) for c in range(4)] + [oproj(t) for t in range(4)]

        for piece in prep_pieces(0):
            piece()
        prevAO = None
        for tc in range(4):
            # pending work interleaved into this round's head stream: next
            # round's projections + previous round's output projection
            pend = []
            if tc < 3:
                pend += prep_pieces(tc + 1)
            if prevAO is not None:
                pend += posts_pieces(tc - 1, prevAO)
            AO = aop.tile([P, 4, DH], F16, tag="AO")
            aotrs3 = oprojs3 = None
            if tc == 3:
                p3 = posts_pieces(3, AO)
                aotrs3, oprojs3 = p3[:4], p3[4:]
            prev_pts = None
            for h in range(HL):
                pts = head_block(AO, h, tc)
                if prev_pts is not None:
                    emit_av(AO, h - 1, tc, prev_pts)
                    if tc == 3 and (h - 1) % 4 == 3:
                        aotrs3[(h - 1) // 4]()
                prev_pts = pts
                for _ in range(2):
                    if pend:
                        pend.pop(0)()
            emit_av(AO, HL - 1, tc, prev_pts)
            for piece in pend:
                piece()
            if tc == 0 and "ao_d" in d:
                nc.sync.dma_start(
                    d["ao_d"].rearrange("p (j e) -> p j e", j=4), AO[:]
                )
            if tc == 3:
                aotrs3[3]()
                for piece in oprojs3:
                    piece()
            prevAO = AO

        if "q8_d" in d:
            nc.sync.dma_start(
                d["q8_d"].rearrange("p (s i t) -> p s i t", s=4, i=2), Q8[:]
            )
            nc.sync.dma_start(d["k8_d"].rearrange("p (i t) -> p i t", i=2), K8[:])
            nc.sync.dma_start(
                d["v_d"].rearrange("p (a b c) -> p a b c", a=16, b=4), V[:]
            )


_NC_CACHE = None


def _build():
    global _NC_CACHE
    if _NC_CACHE is not None:
        return _NC_CACHE
    nc = bacc.Bacc("TRN2", target_bir_lowering=False, debug=False, num_devices=8)
    d = {
        "xt": nc.dram_tensor("xt", (C, T), F16, kind="ExternalInput"),
        "wq": nc.dram_tensor("wq", (C, DH), F16, kind="ExternalInput"),
        "wk": nc.dram_tensor("wk", (C, P), F16, kind="ExternalInput"),
        "wv": nc.dram_tensor("wv", (C, P), F16, kind="ExternalInput"),
        "wo": nc.dram_tensor("wo", (DH, C), F16, kind="ExternalInput"),
        "identh": nc.dram_tensor("identh", (P, P), F16, kind="ExternalInput"),
        "maskc": nc.dram_tensor("maskc", (P, P), F16, kind="ExternalInput"),
        "out": nc.dram_tensor("out", (T, C), F16, kind="ExternalOutput"),
    }
    with tile.TileContext(nc) as tc_:
        _trace(tc_, {k: v[:] for k, v in d.items()})
    nc.compile()
    _NC_CACHE = nc
    return nc


def _in_maps(x, Wq, Wk, Wv, Wo):
    identh = np.eye(P, dtype=np.float16)
    # maskc[p, j] = 0 where q-local j < kpos-local p (strict causal mask),
    # else 1; multiplied into P^T diagonal blocks post-exp
    maskc = np.where(
        np.arange(P)[None, :] < np.arange(P)[:, None], 0.0, 1.0
    ).astype(np.float16)
    maps = []
    for core in range(8):
        b, hf = core // 2, core % 2
        # Wq cols: slot-major [s, g, d] ordering
        qcols = []
        for s in range(4):
            for g in range(4 if s < 2 else 3):
                hloc = g * 4 + s if g < 3 else 12 + s
                H = HEADS_HALF[hf][hloc]
                qcols.extend(range(32 * H, 32 * H + 32))
        # Wk/Wv cols: group-major [g, d]
        kcols = np.concatenate(
            [np.arange(32 * kv, 32 * kv + 32) for kv in KV_HALF[hf]]
        )
        # Wo rows: local-head-major [h, d]
        orows = np.concatenate(
            [np.arange(32 * H, 32 * H + 32) for H in HEADS_HALF[hf]]
        )
        maps.append(
            {
                "xt": np.ascontiguousarray(x[b].T).astype(np.float16),
                "wq": np.ascontiguousarray(Wq[:, qcols] * WSCALE).astype(np.float16),
                "wk": np.ascontiguousarray(Wk[:, kcols] * WSCALE).astype(np.float16),
                "wv": np.ascontiguousarray(Wv[:, kcols]).astype(np.float16),
                "wo": np.ascontiguousarray(Wo[orows, :]).astype(np.float16),
                "identh": identh,
                "maskc": maskc,
            }
        )
    return maps


def run(x, Wq, Wk, Wv, Wo, trace=False):
    nc = _build()
    res = run_bass_kernel_spmd(
        nc, _in_maps(x, Wq, Wk, Wv, Wo), core_ids=list(range(8)), trace=trace
    )
    outs = [r["out"] for r in res.results]
    final = np.empty((4, T, C), np.float32)
    for b in range(4):
        final[b] = outs[2 * b].astype(np.float32) + outs[2 * b + 1].astype(
            np.float32
        )
    return final, res


def kernel(x, Wq, Wk, Wv, Wo):
    x = np.asarray(x, dtype=np.float32)
    out, _ = run(
        x,
        np.asarray(Wq, np.float32),
        np.asarray(Wk, np.float32),
        np.asarray(Wv, np.float32),
        np.asarray(Wo, np.float32),
    )
    return out
